# revision 5
# baseline (speedup 1.0000x reference)
"""Trainium2 Bass kernel for nn_Decoder (2-layer LSTM decoder, B=128, SEQ=256).

Strategy: tensor-parallel over the hidden dimension across 8 cores.
 - Each core owns a 128-wide hidden slice of both LSTM layers' gates
   (gate chunk order [i|f|o|g] so sigmoid covers one contiguous block).
 - The torch-bug-compatible (L,B,2H).reshape(B,-1) in the reference mixes
   batches: fc_in[b] = [h_l[2b], c_l[2b], h_l[2b+1], c_l[2b+1]], l=b//64.
   We keep a per-step "fcT" SBUF tensor laid out in even/odd-batch groups so
   the fc (W_map) matmul runs with M=128 natural batch output.
 - Per step: 2 AllGathers (layer0 h/c slices mid-step, layer1 at step end)
   rebuild the full transposed state on every core.
 - All matmuls in float32r (tf32-class, full PE rate at N>=256).
 - W_map replicated on all cores (out_t computed fully everywhere).
"""
import time
import numpy as np

import jax
from jax.sharding import Mesh, PartitionSpec
from jax.experimental.shard_map import shard_map

from concourse import bass, bacc, mybir, tile
from concourse.bass2jax import (
    _bass_exec_p, install_neuronx_cc_hook, partition_id_tensor,
)

HID = 1024
LAT = 128
INP = 512
L = 2
B = 128
NCORES = 8

f32 = mybir.dt.float32
f32r = mybir.dt.float32r
SIG = mybir.ActivationFunctionType.Sigmoid
TANH = mybir.ActivationFunctionType.Tanh


# ----------------------------------------------------------------------------
# SPMD runner (builds the sharded jitted executable once per Bass module)
# ----------------------------------------------------------------------------
class SpmdRunner:
    def __init__(self, nc, n_cores=NCORES):
        install_neuronx_cc_hook()
        self.nc = nc
        self.n_cores = n_cores
        partition_name = nc.partition_id_tensor.name if nc.partition_id_tensor else None

        in_names, out_names, out_avals, zero_outs = [], [], [], []
        for alloc in nc.m.functions[0].allocations:
            if not isinstance(alloc, mybir.MemoryLocationSet):
                continue
            name = alloc.memorylocations[0].name
            if alloc.kind == "ExternalInput":
                if name != partition_name:
                    in_names.append(name)
            elif alloc.kind == "ExternalOutput":
                out_names.append(name)
                shape = tuple(alloc.tensor_shape)
                dtype = mybir.dt.np(alloc.dtype)
                out_avals.append(jax.core.ShapedArray(shape, dtype))
                zero_outs.append(np.zeros(shape, dtype))
        n_params = len(in_names)
        self.in_names = list(in_names)
        self.out_names = out_names
        self.out_avals = out_avals
        self.zero_outs = zero_outs
        self.n_params = n_params
        all_in_names = in_names + out_names
        if partition_name is not None:
            all_in_names.append(partition_name)

        def _body(*args):
            operands = list(args)
            if partition_name is not None:
                operands.append(partition_id_tensor())
            outs = _bass_exec_p.bind(
                *operands,
                out_avals=tuple(out_avals),
                in_names=tuple(all_in_names),
                out_names=tuple(out_names),
                lowering_input_output_aliases=(),
                sim_require_finite=True,
                sim_require_nnan=True,
                nc=nc,
            )
            return tuple(outs)

        devices = jax.devices()[:n_cores]
        mesh = Mesh(np.asarray(devices), ("core",))
        in_specs = (PartitionSpec("core"),) * (n_params + len(out_names))
        out_specs = (PartitionSpec("core"),) * len(out_names)
        self.fn = jax.jit(
            shard_map(_body, mesh=mesh, in_specs=in_specs,
                      out_specs=out_specs, check_rep=False),
            keep_unused=True,
        )

    def run(self, in_maps, reps=1):
        n_cores = self.n_cores
        per_core = [[np.asarray(m[name]) for name in self.in_names] for m in in_maps]
        concat_in = [
            np.concatenate([per_core[c][i] for c in range(n_cores)], axis=0)
            for i in range(self.n_params)
        ]
        concat_zeros = [
            np.zeros((n_cores * z.shape[0], *z.shape[1:]), z.dtype)
            for z in self.zero_outs
        ]
        times = []
        out_arrs = None
        for _ in range(reps):
            t0 = time.perf_counter()
            out_arrs = self.fn(*concat_in, *concat_zeros)
            jax.block_until_ready(out_arrs)
            times.append(time.perf_counter() - t0)
        results = [
            {
                name: np.asarray(out_arrs[i]).reshape(
                    n_cores, *self.out_avals[i].shape)[c]
                for i, name in enumerate(self.out_names)
            }
            for c in range(n_cores)
        ]
        return results, times


# ----------------------------------------------------------------------------
# Kernel builder
# ----------------------------------------------------------------------------
def build_decoder(seq: int):
    nc = bacc.Bacc("TRN2", target_bir_lowering=False, debug=False,
                   num_devices=NCORES)

    # DRAM I/O (per core). Weight slices differ per core; rest replicated.
    d_wmapT = nc.dram_tensor("wmapT", [4096, 512], f32, kind="ExternalInput")
    d_wih0 = nc.dram_tensor("wih0", [512, 512], f32, kind="ExternalInput")
    d_whh0 = nc.dram_tensor("whh0", [1024, 512], f32, kind="ExternalInput")
    d_wih1 = nc.dram_tensor("wih1", [1024, 512], f32, kind="ExternalInput")
    d_whh1 = nc.dram_tensor("whh1", [1024, 512], f32, kind="ExternalInput")
    d_b0 = nc.dram_tensor("b0", [1, 512], f32, kind="ExternalInput")
    d_b1 = nc.dram_tensor("b1", [1, 512], f32, kind="ExternalInput")
    d_bmap = nc.dram_tensor("bmap", [1, 512], f32, kind="ExternalInput")
    d_ones = nc.dram_tensor("ones", [1, 128], f32, kind="ExternalInput")
    d_ident = nc.dram_tensor("ident", [128, 128], f32, kind="ExternalInput")
    d_fct0 = nc.dram_tensor("fct0", [4096, 128], f32, kind="ExternalInput")
    d_h0T0 = nc.dram_tensor("h0T0", [1024, 128], f32, kind="ExternalInput")
    d_h1T0 = nc.dram_tensor("h1T0", [1024, 128], f32, kind="ExternalInput")
    d_c0own = nc.dram_tensor("c0own", [128, 128], f32, kind="ExternalInput")
    d_c1own = nc.dram_tensor("c1own", [128, 128], f32, kind="ExternalInput")
    d_y = nc.dram_tensor("y", [seq, 128, 512], f32, kind="ExternalOutput")

    with tile.TileContext(nc) as tc:
        with tc.tile_pool(name="wp", bufs=1) as wp, \
             tc.tile_pool(name="fct", bufs=2) as fctp, \
             tc.tile_pool(name="hT", bufs=2) as hTp, \
             tc.tile_pool(name="work", bufs=2) as work, \
             tc.tile_pool(name="tmp", bufs=4) as tmpp, \
             tc.tile_pool(name="cown", bufs=2) as cownp, \
             tc.tile_pool(name="psA", bufs=1, space="PSUM") as psA, \
             tc.tile_pool(name="psB", bufs=2, space="PSUM") as psB, \
             tc.tile_pool(name="dramL", bufs=2, space="DRAM") as dramL:

            # ---------------- init: load + round weights -----------------
            wmapT_sb = wp.tile([128, 32, 512], f32r, name="wmapT_sb")
            wih0_sb = wp.tile([128, 4, 512], f32r, name="wih0_sb")
            whh0_sb = wp.tile([128, 8, 512], f32r, name="whh0_sb")
            wih1_sb = wp.tile([128, 8, 512], f32r, name="wih1_sb")
            whh1_sb = wp.tile([128, 8, 512], f32r, name="whh1_sb")
            b0_r = wp.tile([1, 512], f32r, name="b0_r")
            b1_r = wp.tile([1, 512], f32r, name="b1_r")
            bmap_r = wp.tile([1, 512], f32r, name="bmap_r")
            ones_r = wp.tile([1, 128], f32r, name="ones_r")
            ident_sb = wp.tile([128, 128], f32, name="ident_sb")

            with tc.tile_pool(name="stage", bufs=3) as stp:
                def staged(dst_ap, src_ap, shape=(128, 512)):
                    st = stp.tile(list(shape), f32, name="st", tag="st")
                    nc.sync.dma_start(st[:], src_ap)
                    nc.vector.tensor_copy(dst_ap, st[:])

                def load_rounded(dst_sb, dram_t, ntiles):
                    src = dram_t.ap().rearrange("(k p) n -> p k n", p=128)
                    for k in range(ntiles):
                        staged(dst_sb[:, k, :], src[:, k, :])

                load_rounded(wmapT_sb, d_wmapT, 32)
                load_rounded(wih0_sb, d_wih0, 4)
                load_rounded(whh0_sb, d_whh0, 8)
                load_rounded(wih1_sb, d_wih1, 8)
                load_rounded(whh1_sb, d_whh1, 8)
                staged(b0_r[:], d_b0.ap(), (1, 512))
                staged(b1_r[:], d_b1.ap(), (1, 512))
                staged(bmap_r[:], d_bmap.ap(), (1, 512))
                staged(ones_r[:], d_ones.ap(), (1, 128))
                nc.sync.dma_start(ident_sb[:], d_ident.ap())

                # initial state tiles
                fct_cur = fctp.tile([128, 4, 8, 128], f32r, name="fct", tag="fct")
                src_f = d_fct0.ap().rearrange("(g k p) b -> p g k b", g=4, k=8)
                for g in range(4):
                    for j in range(2):
                        staged(fct_cur[:, g, 4 * j:4 * j + 4, :],
                               src_f[:, g, 4 * j:4 * j + 4, :],
                               (128, 4, 128))

                h0T_cur = hTp.tile([128, 8, 128], f32r, name="h0T", tag="h0T")
                h1T_cur = hTp.tile([128, 8, 128], f32r, name="h1T", tag="h1T")
                src_h0 = d_h0T0.ap().rearrange("(k p) b -> p k b", p=128)
                src_h1 = d_h1T0.ap().rearrange("(k p) b -> p k b", p=128)
                for j in range(2):
                    staged(h0T_cur[:, 4 * j:4 * j + 4, :],
                           src_h0[:, 4 * j:4 * j + 4, :], (128, 4, 128))
                    staged(h1T_cur[:, 4 * j:4 * j + 4, :],
                           src_h1[:, 4 * j:4 * j + 4, :], (128, 4, 128))

                c0own_cur = cownp.tile([128, 128], f32, name="c0own", tag="c0")
                c1own_cur = cownp.tile([128, 128], f32, name="c1own", tag="c1")
                nc.sync.dma_start(c0own_cur[:], d_c0own.ap())
                nc.sync.dma_start(c1own_cur[:], d_c1own.ap())

            # ---------------- steps -----------------
            for t in range(seq):
                last = (t == seq - 1)
                fct_next = fctp.tile([128, 4, 8, 128], f32r,
                                     name=f"fct{t+1}", tag="fct")
                h0T_next = hTp.tile([128, 8, 128], f32r,
                                    name=f"h0T{t+1}", tag="h0T")

                # --- fc (W_map): out_t = sigmoid(fc_in @ WmapT + bmap) ---
                ps_map = psA.tile([128, 512], f32, name="ps_map", tag="ps_map")
                first = True
                for g in range(4):
                    for k in range(8):
                        nc.tensor.matmul(
                            ps_map[:],
                            fct_cur[:, g, k, :],
                            wmapT_sb[:, g * 8 + k, :],
                            start=first, stop=False)
                        first = False
                nc.tensor.matmul(ps_map[:], ones_r[:], bmap_r[:],
                                 start=False, stop=True)
                out_sg = work.tile([128, 512], f32, name="out_sg", tag="out_sg")
                nc.scalar.activation(out_sg[:], ps_map[:], SIG)
                nc.sync.dma_start(d_y.ap()[t], out_sg[:])

                # --- transpose out_t -> outT (lhsT for W_ih0) ---
                ps_oT = psB.tile([128, 512], f32, name="ps_oT", tag="ps_oT")
                for j in range(4):
                    nc.tensor.transpose(
                        ps_oT[:, j * 128:(j + 1) * 128],
                        out_sg[:, j * 128:(j + 1) * 128], ident_sb[:])
                outT = work.tile([128, 4, 128], f32r, name="outT", tag="outT")
                nc.vector.tensor_copy(outT[:], ps_oT[:])

                # --- layer 0 gates ---
                ps_g0 = psB.tile([128, 512], f32, name="ps_g0", tag="ps_g")
                for k in range(8):
                    nc.tensor.matmul(ps_g0[:], h0T_cur[:, k, :],
                                     whh0_sb[:, k, :], start=(k == 0), stop=False)
                for k in range(4):
                    nc.tensor.matmul(ps_g0[:], outT[:, k, :],
                                     wih0_sb[:, k, :], start=False, stop=False)
                nc.tensor.matmul(ps_g0[:], ones_r[:], b0_r[:],
                                 start=False, stop=True)

                # --- layer 0 update (gate order [i|f|o|g]) ---
                sg0 = work.tile([128, 512], f32, name="sg0", tag="sg")
                nc.scalar.activation(sg0[:, 0:384], ps_g0[:, 0:384], SIG)
                nc.scalar.activation(sg0[:, 384:512], ps_g0[:, 384:512], TANH)
                t1 = tmpp.tile([128, 128], f32, name="t1", tag="tmp")
                t2 = tmpp.tile([128, 128], f32, name="t2", tag="tmp")
                nc.vector.tensor_mul(t1[:], sg0[:, 128:256], c0own_cur[:])
                nc.vector.tensor_mul(t2[:], sg0[:, 0:128], sg0[:, 384:512])
                c0own_new = cownp.tile([128, 128], f32, name=f"c0own{t+1}", tag="c0")
                nc.vector.tensor_add(c0own_new[:], t1[:], t2[:])
                t3 = tmpp.tile([128, 128], f32, name="t3", tag="tmp")
                nc.scalar.activation(t3[:], c0own_new[:], TANH)
                h0own = tmpp.tile([128, 128], f32, name="h0own", tag="tmp")
                nc.vector.tensor_mul(h0own[:], sg0[:, 256:384], t3[:])

                # --- transpose new (h0, c0) and AllGather across cores ---
                ps_tpA = psB.tile([128, 256], f32, name="ps_tpA", tag="ps_tp")
                nc.tensor.transpose(ps_tpA[:, 0:128], h0own[:], ident_sb[:])
                nc.tensor.transpose(ps_tpA[:, 128:256], c0own_new[:], ident_sb[:])
                hcA = work.tile([128, 2, 128], f32r, name="hcA", tag="hc")
                nc.vector.tensor_copy(hcA[:, 0, :], ps_tpA[:, 0:128])
                nc.vector.tensor_copy(hcA[:, 1, 0:64], ps_tpA[:, 128:256:2])
                nc.vector.tensor_copy(hcA[:, 1, 64:128], ps_tpA[:, 129:256:2])
                contribA = dramL.tile([128, 2, 128], f32r,
                                      name=f"contribA{t}", tag="contribA")
                nc.sync.dma_start(contribA[:], hcA[:])
                gathA = tc.tile([8, 128, 2, 128], f32r,
                                space=bass.MemorySpace.DRAM, addr_space="Shared",
                                name=f"gathA{t}")[0]
                nc.gpsimd.collective_compute(
                    "AllGather", mybir.AluOpType.bypass,
                    replica_groups=[list(range(NCORES))],
                    ins=[contribA.opt()], outs=[gathA.opt()])
                # scatter gathered layer-0 state into fcT(t+1) + natural h0T(t+1)
                nc.sync.dma_start(
                    h0T_next[:], gathA[:, :, 0, :].transpose([1, 0, 2]))
                for p in range(2):
                    # c groups (1 and 3) straight from gathered pi-split c
                    nc.sync.dma_start(
                        fct_next[:, 2 * p + 1, :, 0:64],
                        gathA[:, :, 1, 64 * p:64 * p + 64].transpose([1, 0, 2]))
                    # h groups (0 and 2) from natural h0T via DVE strided copy
                    nc.vector.tensor_copy(
                        fct_next[:, 2 * p, :, 0:64], h0T_next[:, :, p:128:2])

                # --- layer 1 gates ---
                ps_g1 = psB.tile([128, 512], f32, name="ps_g1", tag="ps_g")
                for k in range(8):
                    nc.tensor.matmul(ps_g1[:], h1T_cur[:, k, :],
                                     whh1_sb[:, k, :], start=(k == 0), stop=False)
                for k in range(8):
                    nc.tensor.matmul(ps_g1[:], h0T_next[:, k, :],
                                     wih1_sb[:, k, :], start=False, stop=False)
                nc.tensor.matmul(ps_g1[:], ones_r[:], b1_r[:],
                                 start=False, stop=True)

                # --- layer 1 update ---
                sg1 = work.tile([128, 512], f32, name="sg1", tag="sg")
                nc.scalar.activation(sg1[:, 0:384], ps_g1[:, 0:384], SIG)
                nc.scalar.activation(sg1[:, 384:512], ps_g1[:, 384:512], TANH)
                u1 = tmpp.tile([128, 128], f32, name="u1", tag="tmp")
                u2 = tmpp.tile([128, 128], f32, name="u2", tag="tmp")
                nc.vector.tensor_mul(u1[:], sg1[:, 128:256], c1own_cur[:])
                nc.vector.tensor_mul(u2[:], sg1[:, 0:128], sg1[:, 384:512])
                c1own_new = cownp.tile([128, 128], f32, name=f"c1own{t+1}", tag="c1")
                nc.vector.tensor_add(c1own_new[:], u1[:], u2[:])

                if not last:
                    u3 = tmpp.tile([128, 128], f32, name="u3", tag="tmp")
                    nc.scalar.activation(u3[:], c1own_new[:], TANH)
                    h1own = tmpp.tile([128, 128], f32, name="h1own", tag="tmp")
                    nc.vector.tensor_mul(h1own[:], sg1[:, 256:384], u3[:])

                    h1T_next = hTp.tile([128, 8, 128], f32r,
                                        name=f"h1T{t+1}", tag="h1T")
                    ps_tpB = psB.tile([128, 256], f32, name="ps_tpB", tag="ps_tp")
                    nc.tensor.transpose(ps_tpB[:, 0:128], h1own[:], ident_sb[:])
                    nc.tensor.transpose(ps_tpB[:, 128:256], c1own_new[:], ident_sb[:])
                    hcB = work.tile([128, 2, 128], f32r, name="hcB", tag="hc")
                    nc.vector.tensor_copy(hcB[:, 0, :], ps_tpB[:, 0:128])
                    nc.vector.tensor_copy(hcB[:, 1, 0:64], ps_tpB[:, 128:256:2])
                    nc.vector.tensor_copy(hcB[:, 1, 64:128], ps_tpB[:, 129:256:2])
                    contribB = dramL.tile([128, 2, 128], f32r,
                                          name=f"contribB{t}", tag="contribB")
                    nc.sync.dma_start(contribB[:], hcB[:])
                    gathB = tc.tile([8, 128, 2, 128], f32r,
                                    space=bass.MemorySpace.DRAM,
                                    addr_space="Shared", name=f"gathB{t}")[0]
                    nc.gpsimd.collective_compute(
                        "AllGather", mybir.AluOpType.bypass,
                        replica_groups=[list(range(NCORES))],
                        ins=[contribB.opt()], outs=[gathB.opt()])
                    nc.sync.dma_start(
                        h1T_next[:], gathB[:, :, 0, :].transpose([1, 0, 2]))
                    for p in range(2):
                        nc.sync.dma_start(
                            fct_next[:, 2 * p + 1, :, 64:128],
                            gathB[:, :, 1, 64 * p:64 * p + 64].transpose([1, 0, 2]))
                        nc.vector.tensor_copy(
                            fct_next[:, 2 * p, :, 64:128], h1T_next[:, :, p:128:2])
                    h1T_cur = h1T_next

                fct_cur = fct_next
                h0T_cur = h0T_next
                c0own_cur = c0own_new
                c1own_cur = c1own_new

    nc.compile()
    return nc


# ----------------------------------------------------------------------------
# Host-side prep: shard weights / build initial state / assemble inputs
# ----------------------------------------------------------------------------
def _prep_inputs(latent, W_fc, b_fc, W_map, b_map,
                 W_ih0, W_hh0, b_ih0, b_hh0, W_ih1, W_hh1, b_ih1, b_hh1):
    latent = np.asarray(latent, np.float32)
    # initial fc: dh = latent @ W_fc.T + b_fc, reshaped (L, B, 2H) row-major
    dh = (latent.astype(np.float64) @ np.asarray(W_fc, np.float64).T
          + np.asarray(b_fc, np.float64))
    dh = dh.astype(np.float32).reshape(L, B, 2 * HID)
    h_init = dh[:, :, :HID]      # (2, 128, 1024)
    c_init = dh[:, :, HID:]      # (2, 128, 1024)

    hT = [np.ascontiguousarray(h_init[l].T) for l in range(L)]  # (1024,128)
    cT = [np.ascontiguousarray(c_init[l].T) for l in range(L)]

    # fcT(0): groups [h-ev, c-ev, h-od, c-od]; each (1024, 128):
    #   cols 0:64 from layer0 (batches 2b resp 2b+1), 64:128 from layer1
    def grp(src, par):
        return np.concatenate([src[0][:, par::2], src[1][:, par::2]], axis=1)
    fct0 = np.stack([grp(hT, 0), grp(cT, 0), grp(hT, 1), grp(cT, 1)])
    fct0 = np.ascontiguousarray(fct0.reshape(4096, 128), np.float32)

    gate_off = [0, 1, 3, 2]   # [i, f, o, g] in torch's [i,f,g,o] layout
    WmapT = np.ascontiguousarray(np.asarray(W_map, np.float32).T)  # (4096,512)

    in_maps = []
    for r in range(NCORES):
        sl = slice(128 * r, 128 * r + 128)

        def gslT(W):
            W = np.asarray(W, np.float32)
            rows = np.concatenate([W[o * HID + 128 * r: o * HID + 128 * r + 128, :]
                                   for o in gate_off], axis=0)
            return np.ascontiguousarray(rows.T)

        def bsl(ba, bb):
            s = np.asarray(ba, np.float32) + np.asarray(bb, np.float32)
            return np.concatenate([s[o * HID + 128 * r: o * HID + 128 * r + 128]
                                   for o in gate_off]).reshape(1, 512)

        in_maps.append({
            "wmapT": WmapT,
            "wih0": gslT(W_ih0),
            "whh0": gslT(W_hh0),
            "wih1": gslT(W_ih1),
            "whh1": gslT(W_hh1),
            "b0": bsl(b_ih0, b_hh0),
            "b1": bsl(b_ih1, b_hh1),
            "bmap": np.asarray(b_map, np.float32).reshape(1, 512),
            "ones": np.ones((1, 128), np.float32),
            "ident": np.eye(128, dtype=np.float32),
            "fct0": fct0,
            "h0T0": hT[0].astype(np.float32),
            "h1T0": hT[1].astype(np.float32),
            "c0own": np.ascontiguousarray(c_init[0][:, sl]),
            "c1own": np.ascontiguousarray(c_init[1][:, sl]),
        })
    return in_maps


_CACHE = {}


def _get_runner(seq: int):
    if seq not in _CACHE:
        nc = build_decoder(seq)
        _CACHE[seq] = SpmdRunner(nc, NCORES)
    return _CACHE[seq]


def kernel(latent, seq_length, W_fc, b_fc, W_map, b_map,
           W_ih0, W_hh0, b_ih0, b_hh0, W_ih1, W_hh1, b_ih1, b_hh1):
    seq = int(seq_length)
    runner = _get_runner(seq)
    in_maps = _prep_inputs(latent, W_fc, b_fc, W_map, b_map,
                           W_ih0, W_hh0, b_ih0, b_hh0,
                           W_ih1, W_hh1, b_ih1, b_hh1)
    results, _ = runner.run(in_maps, reps=1)
    return np.asarray(results[0]["y"], np.float32)


if __name__ == "__main__":
    # smoke test with random inputs at short seq
    rng = np.random.default_rng(0)
    s = 0.05
    inputs = dict(
        latent=rng.standard_normal((B, LAT)).astype(np.float32),
        seq_length=4,
        W_fc=(rng.standard_normal((HID * L * 2, LAT)) * s).astype(np.float32),
        b_fc=(rng.standard_normal((HID * L * 2,)) * s).astype(np.float32),
        W_map=(rng.standard_normal((INP, HID * L * 2)) * s).astype(np.float32),
        b_map=(rng.standard_normal((INP,)) * s).astype(np.float32),
        W_ih0=(rng.standard_normal((4 * HID, INP)) * s).astype(np.float32),
        W_hh0=(rng.standard_normal((4 * HID, HID)) * s).astype(np.float32),
        b_ih0=(rng.standard_normal((4 * HID,)) * s).astype(np.float32),
        b_hh0=(rng.standard_normal((4 * HID,)) * s).astype(np.float32),
        W_ih1=(rng.standard_normal((4 * HID, HID)) * s).astype(np.float32),
        W_hh1=(rng.standard_normal((4 * HID, HID)) * s).astype(np.float32),
        b_ih1=(rng.standard_normal((4 * HID,)) * s).astype(np.float32),
        b_hh1=(rng.standard_normal((4 * HID,)) * s).astype(np.float32),
    )
    out = kernel(**inputs)
    print("out shape:", out.shape, "finite:", np.isfinite(out).all())
